# revision 7
# baseline (speedup 1.0000x reference)
"""Trainium2 Bass kernel for JanusCrossAttention (GQA attention block).

Problem: B=2, S=2048, Q_DIM=KV_DIM=2048, 16 q heads / 4 kv heads, head_dim=128,
qk RMSNorm (eps 1e-5) before causal SDPA, output projection wo.

Sharding over 8 NeuronCores: core c = (b, g) with b = c // 4 (batch),
g = c % 4 (head group = 4 consecutive q heads + their shared kv head g).
Each core computes partial_out[b] = attn_heads(4g..4g+3) @ wo[512g:512g+512, :];
the host sums the 4 partials per batch.

On-device layout strategy (all matmuls contract over the partition axis):
  - host supplies q_stream[b].T / kv_stream[b].T so projections need no
    on-device transpose of the big streams,
  - scores are computed transposed (S^T[k, q] = xkT.T @ xqT) so that
    P^T = exp(S^T) feeds the PV matmul directly (no probability transpose),
  - softmax denominators via an all-ones stationary matmul (gives the row
    sums replicated over all 128 partitions for free),
  - 1/x and rsqrt via exp(-ln x) / exp(-0.5 ln x) (single ACT table set),
  - fp32r (tf32-class) matmul inputs, fp32 PSUM accumulation.
"""

import math
import os
import sys

import numpy as np

_RL = "/opt/trn_rl_repo"
_PP = "/opt/pypackages"
for _p in (_RL, _PP):
    if os.path.isdir(_p) and _p not in sys.path:
        sys.path.append(_p)

from contextlib import ExitStack

import concourse.bass as bass
import concourse.tile as tile
from concourse import bacc, mybir
from concourse.bass_utils import run_bass_kernel_spmd

B, S = 2, 2048
EDIM = 2048            # q/kv stream feature dim
HG = 4                 # q heads per core (head group)
D = 128                # head dim
P = 128                # partitions
NE = EDIM // P         # 16 e-chunks
NSC = S // P           # 16 s-chunks
NST = S // 512         # 4 s-tiles of 512
EPS = 1e-5

f32 = mybir.dt.float32
f32r = mybir.dt.float32r
AF = mybir.ActivationFunctionType

_CACHED = {}


def _build(norm_w_ones: bool):
    nc = bacc.Bacc("TRN2", target_bir_lowering=False, debug=False,
                   enable_asserts=False, num_devices=8)
    dr = lambda name, shape, dt=f32r: nc.dram_tensor(name, shape, dt, kind="ExternalInput").ap()
    qT = dr("qT", [EDIM, S])
    kvT = dr("kvT", [EDIM, S])
    wq = dr("wq", [EDIM, HG * D])
    wkv = dr("wkv", [EDIM, 2 * D])
    wo = dr("wo", [HG * D, S])
    tri = dr("tri", [P, P])          # tri[p, c] = 1 if p <= c
    m2 = dr("m2", [P, 256])          # [zeros(128) | tri]
    onesm = dr("onesm", [P, P])
    ident = dr("ident", [P, P])
    qw = dr("qw", [P, 1], f32)       # q_norm_w as column (used only if not ones)
    kwr = dr("kwr", [1, P], f32)     # k_norm_w as row   (used only if not ones)
    out = nc.dram_tensor("out", [S, S], f32, kind="ExternalOutput").ap()

    qT_r = qT.rearrange("(c p) s -> p c s", p=P)
    kvT_r = kvT.rearrange("(c p) s -> p c s", p=P)
    wq_r = wq.rearrange("(c p) d -> p c d", p=P)
    wkv_r = wkv.rearrange("(c p) d -> p c d", p=P)
    wo_r = wo.rearrange("(c p) o -> p c o", p=P)

    with tile.TileContext(nc) as tc, ExitStack() as ctx:
        # ---- persistent SBUF ----
        pers = ctx.enter_context(tc.tile_pool(name="pers", bufs=1))
        xqTn = pers.tile([P, HG, S], f32r)       # normalized q^T per head
        xkTn = pers.tile([P, S], f32r)           # normalized k^T
        v_sb = pers.tile([P, NSC, D], f32r)      # v, natural [s-chunk, d]
        attnT = pers.tile([P, HG, S], f32r)      # attn output^T (normalized)
        tri_sb = pers.tile([P, P], f32r)
        m2_sb = pers.tile([P, 256], f32r)
        ones_sb = pers.tile([P, P], f32r)
        id_sb = pers.tile([P, P], f32r)
        eps_b = pers.tile([P, 1], f32)
        exq_b = pers.tile([P, 1], f32)           # -0.5*ln(128): folds 1/sqrt(d) into rq
        ksq = pers.tile([P, NE], f32)            # k sumsq columns per s-chunk
        rkl = pers.tile([P, NE], f32)            # ln of mean+eps
        rkc = pers.tile([P, NE], f32)            # rsqrt(mean+eps) columns
        nc.sync.dma_start(tri_sb[:], tri[:])
        nc.sync.dma_start(m2_sb[:], m2[:])
        nc.sync.dma_start(ones_sb[:], onesm[:])
        nc.sync.dma_start(id_sb[:], ident[:])
        nc.vector.memset(eps_b[:], EPS)
        nc.vector.memset(exq_b[:], -0.5 * math.log(128.0))
        if not norm_w_ones:
            qw_sb = pers.tile([P, 1], f32)
            kw_sb = pers.tile([P, P], f32)
            nc.sync.dma_start(qw_sb[:], qw[:])
            nc.sync.dma_start(kw_sb[:], kwr[:].to_broadcast((P, P)))

        # ================= Phase A1: k/v projections =================
        # kv natural: out[s128, 256] = sum_e kvT[e, s].T @ wkv[e, :]
        with ExitStack() as c1:
            kvt_pool = c1.enter_context(tc.tile_pool(name="kvt", bufs=3))
            ps_kv = c1.enter_context(tc.tile_pool(name="ps_kv", bufs=2, space="PSUM"))
            ps_kt = c1.enter_context(tc.tile_pool(name="ps_kt", bufs=2, space="PSUM"))
            sc1 = c1.enter_context(tc.tile_pool(name="sc1", bufs=3))
            wkv_pool = c1.enter_context(tc.tile_pool(name="wkvp", bufs=1))
            wkv_sb = wkv_pool.tile([P, NE, 2 * D], f32r)
            nc.sync.dma_start(wkv_sb[:], wkv_r[:])
            for sc in range(NSC):
                kvt = kvt_pool.tile([P, NE, P], f32r, tag="kvt")
                nc.sync.dma_start(kvt[:], kvT_r[:, :, sc * P:(sc + 1) * P])
                pkv = ps_kv.tile([P, 2 * D], f32, tag="pkv")
                for e in range(NE):
                    nc.tensor.matmul(pkv[:], kvt[:, e, :], wkv_sb[:, e, :],
                                     start=(e == 0), stop=(e == NE - 1))
                # k sumsq along free (head_dim) via Square + accum_out
                sqk = sc1.tile([P, D], f32, tag="sqk")
                nc.scalar.activation(sqk[:], pkv[:, 0:D], AF.Square,
                                     accum_out=ksq[:, sc:sc + 1])
                # v eviction
                nc.vector.tensor_copy(v_sb[:, sc, :], pkv[:, D:2 * D])
                # k eviction happens after rk is ready (below) -> stash psum ref
                # normalize k with per-partition rk = rsqrt(mean+eps) = exp(-0.5 ln(.))
                nc.scalar.activation(rkl[:, sc:sc + 1], ksq[:, sc:sc + 1], AF.Ln,
                                     scale=1.0 / D, bias=eps_b[:])
                nc.scalar.activation(rkc[:, sc:sc + 1], rkl[:, sc:sc + 1], AF.Exp,
                                     scale=-0.5)
                kn = sc1.tile([P, D], f32r, tag="kn")
                nc.vector.tensor_scalar_mul(kn[:], pkv[:, 0:D], rkc[:, sc:sc + 1])
                if not norm_w_ones:
                    nc.vector.tensor_mul(kn[:], kn[:], kw_sb[:, 0:D])
                # transpose k chunk -> xkTn columns
                pkt = ps_kt.tile([P, P], f32r, tag="pkt")
                nc.tensor.transpose(pkt[:], kn[:], id_sb[:])
                nc.vector.tensor_copy(xkTn[:, sc * P:(sc + 1) * P], pkt[:])

        # ================= Phase A2: q projection =================
        with ExitStack() as c2:
            wq_pool = c2.enter_context(tc.tile_pool(name="wqp", bufs=1))
            wq_sb = wq_pool.tile([P, NE, HG * D], f32r)
            nc.sync.dma_start(wq_sb[:], wq_r[:])
            qt_pool = c2.enter_context(tc.tile_pool(name="qtp", bufs=3))
            ps_q = c2.enter_context(tc.tile_pool(name="ps_q", bufs=4, space="PSUM"))
            ps_s = c2.enter_context(tc.tile_pool(name="ps_s", bufs=2, space="PSUM"))
            sc2 = c2.enter_context(tc.tile_pool(name="sc2", bufs=3))
            for st in range(NST):
                ssl = slice(st * 512, (st + 1) * 512)
                qta = qt_pool.tile([P, NE // 2, 512], f32r, tag="qt")
                nc.sync.dma_start(qta[:], qT_r[:, 0:NE // 2, ssl])
                qtb = qt_pool.tile([P, NE // 2, 512], f32r, tag="qt")
                nc.sync.dma_start(qtb[:], qT_r[:, NE // 2:NE, ssl])
                for h in range(HG):
                    pq = ps_q.tile([P, 512], f32, tag="pq")
                    for e in range(NE):
                        qt = qta if e < NE // 2 else qtb
                        nc.tensor.matmul(pq[:], wq_sb[:, e, h * D:(h + 1) * D],
                                         qt[:, e % (NE // 2), :],
                                         start=(e == 0), stop=(e == NE - 1))
                    sq = sc2.tile([P, 512], f32r, tag="sq")
                    nc.scalar.activation(sq[:], pq[:], AF.Square)
                    pss = ps_s.tile([P, 512], f32, tag="pss")
                    nc.tensor.matmul(pss[:], ones_sb[:], sq[:], start=True, stop=True)
                    lq = sc2.tile([P, 512], f32, tag="lq")
                    nc.scalar.activation(lq[:], pss[:], AF.Ln, scale=1.0 / D, bias=eps_b[:])
                    rq = sc2.tile([P, 512], f32, tag="rq")
                    nc.scalar.activation(rq[:], lq[:], AF.Exp, scale=-0.5, bias=exq_b[:])
                    nc.vector.tensor_mul(xqTn[:, h, ssl], pq[:], rq[:])
                    if not norm_w_ones:
                        nc.vector.tensor_scalar_mul(xqTn[:, h, ssl], xqTn[:, h, ssl],
                                                    qw_sb[:])

        # ================= Phase B: attention =================
        with ExitStack() as c3:
            ps_sc = c3.enter_context(tc.tile_pool(name="ps_sc", bufs=3, space="PSUM"))
            ps_pv = c3.enter_context(tc.tile_pool(name="ps_pv", bufs=2, space="PSUM"))
            ps_dn = c3.enter_context(tc.tile_pool(name="ps_dn", bufs=2, space="PSUM"))
            ptp = c3.enter_context(tc.tile_pool(name="ptp", bufs=6))
            sc3 = c3.enter_context(tc.tile_pool(name="sc3", bufs=3))
            for j in range(NST):            # q tile (512 wide)
                jsl = slice(j * 512, (j + 1) * 512)
                nkc = 4 * j + 4             # live k chunks
                for h in range(HG):
                    ppv = ps_pv.tile([P, 512], f32, tag="ppv")
                    pdn = ps_dn.tile([P, 512], f32, tag="pdn")
                    for kc in range(nkc):
                        r = kc - 4 * j      # <0: full chunk
                        if r < 0 or r == 0:
                            o = 0
                        elif r == 1:
                            o = 128
                        else:
                            o = 256
                        w = 512 - o
                        psc = ps_sc.tile([P, 512], f32, tag="psc")
                        nc.tensor.matmul(psc[:, o:512], xkTn[:, kc * P:(kc + 1) * P],
                                         xqTn[:, h, j * 512 + o:(j + 1) * 512],
                                         start=True, stop=True)
                        pt = ptp.tile([P, 512], f32r, tag="pt")
                        nc.scalar.activation(pt[:, o:512], psc[:, o:512], AF.Exp)
                        if r == 0:
                            nc.vector.tensor_mul(pt[:, 0:128], pt[:, 0:128], tri_sb[:])
                        elif r == 1:
                            nc.vector.tensor_mul(pt[:, 128:256], pt[:, 128:256], tri_sb[:])
                        elif r == 2:
                            nc.vector.tensor_mul(pt[:, 256:384], pt[:, 256:384], tri_sb[:])
                        elif r == 3:
                            nc.vector.tensor_mul(pt[:, 256:512], pt[:, 256:512], m2_sb[:])
                        first, last = (kc == 0), (kc == nkc - 1)
                        nc.tensor.matmul(ppv[:, o:512], v_sb[:, kc, :], pt[:, o:512],
                                         start=first, stop=last)
                        nc.tensor.matmul(pdn[:, o:512], ones_sb[:], pt[:, o:512],
                                         start=first, stop=last)
                    lc = sc3.tile([P, 512], f32, tag="lc")
                    nc.scalar.activation(lc[:], pdn[:], AF.Ln)
                    rc = sc3.tile([P, 512], f32, tag="rc")
                    nc.scalar.activation(rc[:], lc[:], AF.Exp, scale=-1.0)
                    nc.vector.tensor_mul(attnT[:, h, jsl], ppv[:], rc[:])

        # ================= Phase C: output projection =================
        with ExitStack() as c4:
            wo_pool = c4.enter_context(tc.tile_pool(name="wop", bufs=1))
            wo_sb = wo_pool.tile([P, HG, S], f32r)
            nc.sync.dma_start(wo_sb[:], wo_r[:])
            ps_o = c4.enter_context(tc.tile_pool(name="ps_o", bufs=4, space="PSUM"))
            ob = c4.enter_context(tc.tile_pool(name="ob", bufs=4))
            for sc in range(NSC):
                for ot in range(NST):
                    po = ps_o.tile([P, 512], f32, tag="po")
                    for h in range(HG):
                        nc.tensor.matmul(po[:], attnT[:, h, sc * P:(sc + 1) * P],
                                         wo_sb[:, h, ot * 512:(ot + 1) * 512],
                                         start=(h == 0), stop=(h == HG - 1))
                    os_t = ob.tile([P, 512], f32, tag="os")
                    if (sc * NST + ot) % 2 == 0:
                        nc.scalar.copy(os_t[:], po[:])
                    else:
                        nc.vector.tensor_copy(os_t[:], po[:])
                    nc.sync.dma_start(out[sc * P:(sc + 1) * P, ot * 512:(ot + 1) * 512],
                                      os_t[:])
    nc.compile()
    return nc


def _get_nc(norm_w_ones: bool):
    key = ("nc", norm_w_ones)
    if key not in _CACHED:
        _CACHED[key] = _build(norm_w_ones)
    return _CACHED[key]


def _consts():
    if "consts" not in _CACHED:
        tri = (np.arange(P)[:, None] <= np.arange(P)[None, :]).astype(np.float32)
        m2 = np.concatenate([np.zeros((P, P), np.float32), tri], axis=1)
        _CACHED["consts"] = {
            "tri": tri,
            "m2": m2,
            "onesm": np.ones((P, P), np.float32),
            "ident": np.eye(P, dtype=np.float32),
        }
    return _CACHED["consts"]


def kernel(q_stream, kv_stream, wq, wk, wv, wo, q_norm_w, k_norm_w):
    q_stream = np.ascontiguousarray(np.asarray(q_stream, dtype=np.float32))
    kv_stream = np.ascontiguousarray(np.asarray(kv_stream, dtype=np.float32))
    wq = np.asarray(wq, dtype=np.float32)
    wk = np.asarray(wk, dtype=np.float32)
    wv = np.asarray(wv, dtype=np.float32)
    wo = np.asarray(wo, dtype=np.float32)
    q_norm_w = np.asarray(q_norm_w, dtype=np.float32)
    k_norm_w = np.asarray(k_norm_w, dtype=np.float32)

    ones = bool(np.all(q_norm_w == 1.0) and np.all(k_norm_w == 1.0))
    nc = _get_nc(ones)
    c = _consts()

    qTs = [np.ascontiguousarray(q_stream[b].T) for b in range(B)]
    kvTs = [np.ascontiguousarray(kv_stream[b].T) for b in range(B)]
    in_maps = []
    for core in range(8):
        b, g = core // 4, core % 4
        in_maps.append({
            "qT": qTs[b],
            "kvT": kvTs[b],
            "wq": np.ascontiguousarray(wq[:, 512 * g:512 * (g + 1)]),
            "wkv": np.ascontiguousarray(
                np.concatenate([wk[:, D * g:D * (g + 1)], wv[:, D * g:D * (g + 1)]],
                               axis=1)),
            "wo": np.ascontiguousarray(wo[512 * g:512 * (g + 1), :]),
            "tri": c["tri"],
            "m2": c["m2"],
            "onesm": c["onesm"],
            "ident": c["ident"],
            "qw": q_norm_w.reshape(P, 1),
            "kwr": k_norm_w.reshape(1, P),
        })
    r = run_bass_kernel_spmd(nc, in_maps, core_ids=list(range(8)), trace=False)
    out = np.zeros((B, S, S), dtype=np.float32)
    for core in range(8):
        out[core // 4] += r.results[core]["out"]
    return out
